# revision 5
# baseline (speedup 1.0000x reference)
"""MoE router gate kernel for Trainium2 (Bass/Tile), 8-core data-parallel.

Computes, for x[16384, 7168], weight[256, 7168], bias[256]:
    scores  = sigmoid(x @ weight.T)
    biased  = scores + bias
    indices = top8(biased)                        (descending, int32)
    weights = scores[indices] / sum * 2.5         (float32)

Sharding: data-parallel over tokens (2048 tokens/core), weight/bias
replicated.  Host pre-arranges x into a transposed tiled layout so the
contraction dim lands on SBUF partitions with contiguous DMAs.

Default variant "k3": x is shipped as xh=fp16(x*16) (2B) plus the fp8
residual xl8 (1B) — 3 bytes/element of DMA.  The fp16 main matmul
(1 cyc/row) accumulates xh*wh; a DoubleRow fp8 matmul (0.5 cyc/row)
accumulates the two correction terms fp8(xh)*wl8 + xl8*wh8, where
fp8(xh) is derived on-device by an Activation-engine cast so it costs
no HBM traffic.  Score error ~2^-15: top-8 selection matches the fp32
reference on all but a few of 16384 rows.
"""

import os
from concurrent.futures import ThreadPoolExecutor

import numpy as np

TOKENS = 16384
DIM = 7168
NEXP = 256
TOPK = 8
ROUTE_SCALE = 2.5
NCORES = 8
TPC = TOKENS // NCORES          # tokens per core: 2048
P = 128                         # partitions / tile height
NTILES = TPC // P               # 16 token tiles per core
KC = DIM // P                   # 56 contraction chunks

# Matmul variant:
#   "k3"      fp16 main + fp8 DoubleRow corrections, xh8 cast on-device (3B/elem DMA)
#   "hyb4"    same math, xh8 shipped from host (4B/elem DMA)
#   "fp32"    exact, 4 cyc/row
#   "f32r"    1 cyc/row, tf32-like (fails the index gate; kept for probing)
MM_DTYPE = os.environ.get("GATE_MM_DTYPE", "k3")
X_SCALE = 16.0   # keep x_lo out of fp16-denormal range
W_SCALE = 64.0   # keep w_lo out of fp16-denormal range
S_XL = 512.0     # scale of fp8(x residual)
S_WH = 8.0       # scale of fp8(w)
S_WL = S_XL * S_WH * 1.0  # scale of fp8(w residual); must equal S_XL*S_WH


def _build_program(reps=1):
    import concourse.bacc as bacc
    import concourse.mybir as mybir
    import concourse.tile as tile

    f32 = mybir.dt.float32
    f16 = mybir.dt.float16
    f8 = mybir.dt.float8e4
    u32 = mybir.dt.uint32
    hyb = MM_DTYPE in ("k3", "hyb4")
    cast_dev = MM_DTYPE == "k3"
    mm_dt = None if hyb else {
        "fp32": mybir.dt.float32,
        "f32r": mybir.dt.float32r,
    }[MM_DTYPE]
    sig_scale = 1.0 / (X_SCALE * W_SCALE) if hyb else 1.0

    nc = bacc.Bacc(
        "TRN2",
        target_bir_lowering=False,
        debug=False,
        enable_asserts=False,
        num_devices=NCORES,
    )

    if hyb:
        xh_d = nc.dram_tensor("xh", [NTILES, P, KC, P], f16, kind="ExternalInput").ap()
        # residual fp8: one contiguous [P, KC*P] block per tile
        xl8_d = nc.dram_tensor(
            "xl8", [NTILES, P, KC, P], f8, kind="ExternalInput"
        ).ap()
        if not cast_dev:
            xh8_d = nc.dram_tensor(
                "xh8", [NTILES, P, KC, P], f8, kind="ExternalInput"
            ).ap()
        wh_d = nc.dram_tensor("wh", [P, KC, NEXP], f16, kind="ExternalInput").ap()
        # w8[:, 0] = wl8 (pairs xh8), w8[:, 1] = wh8 (pairs xl8)
        w8_d = nc.dram_tensor("w8", [P, 2, KC, NEXP], f8, kind="ExternalInput").ap()
    else:
        xt_d = nc.dram_tensor(
            "xt", [NTILES, P, KC, P], mm_dt, kind="ExternalInput"
        ).ap()
        wt_d = nc.dram_tensor("wt", [P, KC, NEXP], mm_dt, kind="ExternalInput").ap()
    bb_d = nc.dram_tensor("bb", [P, NEXP], f32, kind="ExternalInput").ap()
    ow_d = nc.dram_tensor("out_w", [NTILES, P, TOPK], f32, kind="ExternalOutput").ap()
    oi_d = nc.dram_tensor("out_i", [NTILES, P, TOPK], u32, kind="ExternalOutput").ap()

    with tile.TileContext(nc) as tc:
        with (
            tc.tile_pool(name="const", bufs=1) as const_pool,
            tc.tile_pool(name="xin", bufs=3) as x_pool,
            tc.tile_pool(name="psum", bufs=3, space="PSUM") as ps_pool,
            tc.tile_pool(name="epi", bufs=3) as ep_pool,
        ):
            if hyb:
                wh_sb = const_pool.tile([P, KC, NEXP], f16)
                nc.sync.dma_start(wh_sb[:], wh_d)
                w8_sb = const_pool.tile([P, 2, KC, NEXP], f8)
                nc.sync.dma_start(w8_sb[:], w8_d)
            else:
                wt_sb = const_pool.tile([P, KC, NEXP], mm_dt)
                nc.sync.dma_start(wt_sb[:], wt_d)
            bb_sb = const_pool.tile([P, NEXP], f32)
            nc.sync.dma_start(bb_sb[:], bb_d)

            seq = [b for _ in range(reps) for b in range(NTILES)]
            loaded = []

            def issue_load(b):
                """DMA (and cast) the inputs for token tile b.  Called one
                iteration ahead so the ACT-engine cast for tile b+1 sits in
                front of tile b's sigmoid in the ACT queue — otherwise the
                next tile's DoubleRow matmuls stall on a cast that is stuck
                behind an epilogue dependent on this tile's matmuls."""
                if hyb:
                    xh_sb = x_pool.tile([P, KC, P], f16, tag="xh")
                    nc.sync.dma_start(xh_sb[:], xh_d[b])
                    # x8[:, 0] = fp8(xh) (pairs wl8), x8[:, 1] = xl8 (pairs wh8)
                    x8_sb = x_pool.tile([P, 2, KC, P], f8, tag="x8")
                    nc.sync.dma_start(x8_sb[:, 1], xl8_d[b])
                    if cast_dev:
                        nc.scalar.activation(
                            x8_sb[:, 0],
                            xh_sb[:],
                            mybir.ActivationFunctionType.Copy,
                        )
                    else:
                        nc.sync.dma_start(x8_sb[:, 0], xh8_d[b])
                    loaded.append((xh_sb, x8_sb))
                else:
                    xt_sb = x_pool.tile([P, KC, P], mm_dt, tag="xt")
                    nc.sync.dma_start(xt_sb[:], xt_d[b])
                    loaded.append((xt_sb,))

            LOOKAHEAD = 2
            for j in range(min(LOOKAHEAD, len(seq))):
                issue_load(seq[j])
            for i, b in enumerate(seq):
                if i + LOOKAHEAD < len(seq):
                    issue_load(seq[i + LOOKAHEAD])
                ps = ps_pool.tile([P, NEXP], f32, tag="ps")
                if hyb:
                    xh_sb, x8_sb = loaded.pop(0)
                    psc = ps_pool.tile([P, NEXP], f32, tag="psc")
                    # fp16 main pass first: the DoubleRow pass needs the
                    # ACT-cast x8 tile, so give the cast maximal slack.
                    for k in range(KC):
                        nc.tensor.matmul(
                            ps[:],
                            xh_sb[:, k, :],
                            wh_sb[:, k, :],
                            start=(k == 0),
                            stop=(k == KC - 1),
                        )
                    for k in range(KC):
                        nc.tensor.matmul(
                            psc[:],
                            x8_sb[:, :, k, :],
                            w8_sb[:, :, k, :],
                            start=(k == 0),
                            stop=(k == KC - 1),
                            perf_mode=mybir.MatmulPerfMode.DoubleRow,
                        )
                else:
                    (xt_sb,) = loaded.pop(0)
                    for k in range(KC):
                        nc.tensor.matmul(
                            ps[:],
                            xt_sb[:, k, :],
                            wt_sb[:, k, :],
                            start=(k == 0),
                            stop=(k == KC - 1),
                        )

                if hyb:
                    # correction /= S_WL, then add main.  Only one PSUM
                    # operand allowed per DVE op: stage psc/S_WL in SBUF.
                    corr = ep_pool.tile([P, NEXP], f32, tag="corr")
                    nc.vector.tensor_scalar(
                        corr[:],
                        psc[:],
                        1.0 / S_WL,
                        None,
                        op0=mybir.AluOpType.mult,
                    )
                    comb = ep_pool.tile([P, NEXP], f32, tag="comb")
                    nc.vector.tensor_add(comb[:], ps[:], corr[:])
                    sig_in = comb
                else:
                    sig_in = ps
                sig = ep_pool.tile([P, NEXP], f32, tag="sig")
                nc.scalar.activation(
                    sig[:],
                    sig_in[:],
                    mybir.ActivationFunctionType.Sigmoid,
                    scale=sig_scale,
                )

                biased = ep_pool.tile([P, NEXP], f32, tag="biased")
                nc.vector.tensor_add(biased[:], sig[:], bb_sb[:])

                max8 = ep_pool.tile([P, TOPK], f32, tag="max8")
                nc.vector.max(out=max8[:], in_=biased[:])
                idx = ep_pool.tile([P, TOPK], u32, tag="idx")
                nc.vector.max_index(out=idx[:], in_max=max8[:], in_values=biased[:])

                # Gather original sigmoid scores at the selected experts:
                # sel[:, j] = sum_e (biased[:, e] == max8[:, j]) * sig[:, e]
                sel = ep_pool.tile([P, TOPK], f32, tag="sel")
                scratch = ep_pool.tile([P, NEXP], f32, tag="scratch")
                for j in range(TOPK):
                    nc.vector.scalar_tensor_tensor(
                        out=scratch[:],
                        in0=biased[:],
                        scalar=max8[:, j : j + 1],
                        in1=sig[:],
                        op0=mybir.AluOpType.is_equal,
                        op1=mybir.AluOpType.mult,
                        accum_out=sel[:, j : j + 1],
                    )

                ssum = ep_pool.tile([P, 1], f32, tag="ssum")
                nc.vector.tensor_reduce(
                    ssum[:], sel[:], axis=mybir.AxisListType.X, op=mybir.AluOpType.add
                )
                rec = ep_pool.tile([P, 1], f32, tag="rec")
                nc.vector.reciprocal(rec[:], ssum[:])

                wout = ep_pool.tile([P, TOPK], f32, tag="wout")
                nc.vector.tensor_scalar(
                    wout[:],
                    sel[:],
                    rec[:],
                    ROUTE_SCALE,
                    op0=mybir.AluOpType.mult,
                    op1=mybir.AluOpType.mult,
                )

                nc.sync.dma_start(ow_d[b], wout[:])
                nc.sync.dma_start(oi_d[b], idx[:])

    nc.compile()
    return nc


def _tile_x(x_shard):
    # [2048, D] -> [16, 128(tok), 56(d_out), 128(d_in)] -> [16, 128(d_in), 56, 128(tok)]
    return x_shard.reshape(NTILES, P, KC, P).transpose(0, 3, 2, 1)


def _prep_core_inputs(x_shard, wt, bb):
    if MM_DTYPE in ("k3", "hyb4"):
        import ml_dtypes

        f8 = ml_dtypes.float8_e4m3
        xs = (x_shard * X_SCALE).astype(np.float32)
        xh = xs.astype(np.float16)
        xl = xs - xh.astype(np.float32)
        out = {
            "xh": np.ascontiguousarray(_tile_x(xh)),
            "xl8": np.ascontiguousarray(_tile_x((xl * S_XL).astype(f8))),
            "wh": wt[0],
            "w8": wt[1],
            "bb": bb,
        }
        if MM_DTYPE == "hyb4":
            out["xh8"] = np.ascontiguousarray(_tile_x(xh.astype(f8)))
        return out
    return {"xt": np.ascontiguousarray(_tile_x(x_shard)), "wt": wt, "bb": bb}


def _prep_all(x, w, bias):
    # weight [256, 7168] -> [128(d_in), 56(d_out), 256(exp)]
    def _tile_w(warr):
        return np.ascontiguousarray(warr.reshape(NEXP, KC, P).transpose(2, 1, 0))

    if MM_DTYPE in ("k3", "hyb4"):
        import ml_dtypes

        f8 = ml_dtypes.float8_e4m3
        ws = (w * W_SCALE).astype(np.float32)
        wh = ws.astype(np.float16)
        wl = ws - wh.astype(np.float32)
        wl8 = _tile_w((wl * S_WL).astype(f8))             # pairs fp8(xh)
        wh8 = _tile_w((ws * S_WH).astype(f8))             # pairs xl8
        w8 = np.ascontiguousarray(np.stack([wl8, wh8], axis=1))
        wt = (_tile_w(wh), w8)
    else:
        wt = _tile_w(w)
    bb = np.ascontiguousarray(np.broadcast_to(bias, (P, NEXP)))

    with ThreadPoolExecutor(NCORES) as pool:
        return list(
            pool.map(
                lambda c: _prep_core_inputs(x[c * TPC : (c + 1) * TPC], wt, bb),
                range(NCORES),
            )
        )


def _collect(results):
    weights = np.concatenate(
        [r["out_w"].reshape(TPC, TOPK) for r in results], axis=0
    ).astype(np.float32)
    indices = np.concatenate(
        [r["out_i"].reshape(TPC, TOPK) for r in results], axis=0
    ).astype(np.int32)
    return weights, indices


def kernel(**inputs):
    from concourse.bass_utils import run_bass_kernel_spmd

    x = np.ascontiguousarray(np.asarray(inputs["x"], dtype=np.float32))
    w = np.ascontiguousarray(np.asarray(inputs["weight"], dtype=np.float32))
    bias = np.asarray(inputs["bias"], dtype=np.float32)

    in_maps = _prep_all(x, w, bias)
    nc = _build_program()
    res = run_bass_kernel_spmd(nc, in_maps, core_ids=list(range(NCORES)), trace=False)
    return _collect(res.results)
